# revision 20
# baseline (speedup 1.0000x reference)
"""CapsNet forward on 8 Trainium2 NeuronCores (batch data parallelism +
4 tiny AllReduces for the cross-batch scalars in dynamic routing)."""
import os
import numpy as np

import concourse.bass as bass
import concourse.bacc as bacc
import concourse.tile as tile
import concourse.mybir as mybir
import concourse.bass_isa as bass_isa
from concourse import bass_utils

F32 = mybir.dt.float32
F32R = mybir.dt.float32r
I32 = mybir.dt.int32
AF = mybir.ActivationFunctionType
ALU = mybir.AluOpType

NCORES = 8
B = 512
BL = B // NCORES
R = 1152
NCAP = 10
O16 = 16
I8 = 8
FLAT = R * I8
KT = FLAT // 128          # 72 K-tiles; tile t=(i,rc): i=t//9, rc=t%9, rows r=rc*128+p
NCHUNK = 8
S = BL // NCHUNK
DEBUG = bool(int(os.environ.get("CAPS_DEBUG", "0")))

_CACHE = {}


def build_nc():
    nc = bacc.Bacc("TRN2", target_bir_lowering=False, debug=False,
                   num_devices=NCORES)

    data = nc.dram_tensor("data", [BL, 1, 28, 28], F32, kind="ExternalInput")
    conv_w = nc.dram_tensor("conv_w", [256, 1, 9, 9], F32, kind="ExternalInput")
    conv_b = nc.dram_tensor("conv_b", [256], F32, kind="ExternalInput")
    pc_w = nc.dram_tensor("pc_w", [256, 256, 9, 9], F32, kind="ExternalInput")
    pc_b = nc.dram_tensor("pc_b", [256], F32, kind="ExternalInput")
    W_dc = nc.dram_tensor("W_dc", [R, NCAP, O16, I8], F32, kind="ExternalInput")
    w1 = nc.dram_tensor("w1", [512, 160], F32, kind="ExternalInput")
    b1 = nc.dram_tensor("b1", [512], F32, kind="ExternalInput")
    w2 = nc.dram_tensor("w2", [1024, 512], F32, kind="ExternalInput")
    b2 = nc.dram_tensor("b2", [1024], F32, kind="ExternalInput")
    w3 = nc.dram_tensor("w3", [784, 1024], F32, kind="ExternalInput")
    b3 = nc.dram_tensor("b3", [784], F32, kind="ExternalInput")
    out_v = nc.dram_tensor("out_v", [BL, NCAP, O16, 1], F32, kind="ExternalOutput")
    out_recon = nc.dram_tensor("out_recon", [BL, 1, 28, 28], F32,
                               kind="ExternalOutput")
    out_masked = nc.dram_tensor("out_masked", [BL, NCAP], F32, kind="ExternalOutput")
    dbg = {}
    if DEBUG:
        dbg["F"] = nc.dram_tensor("dbg_F", [128, 4, R], F32, kind="ExternalOutput")
        dbg["uT"] = nc.dram_tensor("dbg_uT", [128, KT, BL], F32, kind="ExternalOutput")
        dbg["udup"] = nc.dram_tensor("dbg_udup", [128, FLAT], F32,
                                     kind="ExternalOutput")
        dbg["sT"] = nc.dram_tensor("dbg_sT", [3, 160, BL], F32, kind="ExternalOutput")
        dbg["bij"] = nc.dram_tensor("dbg_bij", [2, 128, 5, R], F32,
                                    kind="ExternalOutput")
        dbg["cls"] = nc.dram_tensor("dbg_cls", [NCAP, BL], F32, kind="ExternalOutput")
        dbg["g"] = nc.dram_tensor("dbg_g", [3, 1], F32, kind="ExternalOutput")
        dbg["q"] = nc.dram_tensor("dbg_q", [NCAP, BL], F32, kind="ExternalOutput")

    data_f = data.ap().rearrange("b one h w -> b (one h w)")
    cw_f = conv_w.ap().rearrange("o one kh kw -> o (one kh kw)")
    pcw_f = pc_w.ap().rearrange("o i kh kw -> o (i kh kw)")
    wdc_f = W_dc.ap().rearrange("r c o i -> r (c o i)")
    outv_f = out_v.ap().rearrange("b c o one -> b (c o one)")
    outr_f = out_recon.ap().rearrange("b one h w -> b (one h w)")

    with tile.TileContext(nc) as tc:
        with tc.tile_pool(name="const", bufs=1) as constp, \
             tc.tile_pool(name="dram", bufs=1, space="DRAM") as dramp, \
             tc.tile_pool(name="psA", bufs=3, space="PSUM") as psA, \
             tc.tile_pool(name="psB", bufs=2, space="PSUM") as psB, \
             tc.tile_pool(name="psD", bufs=2, space="PSUM") as psD:

            # ---------------- constants ----------------
            ident_d = nc.inline_tensor(np.eye(128, dtype=np.float32), name="ident_d")
            ident = constp.tile([128, 128], F32)
            nc.sync.dma_start(out=ident, in_=ident_d.ap())
            ones_np = np.zeros((160, NCAP), np.float32)
            for c in range(NCAP):
                ones_np[c * 16:(c + 1) * 16, c] = 1.0
            ones_d = nc.inline_tensor(ones_np, name="ones_d")
            cidx_d = nc.inline_tensor(
                np.arange(NCAP, dtype=np.float32).reshape(NCAP, 1), name="cidx_d")
            ones_lo = constp.tile([128, NCAP], F32)
            ones_hi = constp.tile([32, NCAP], F32)
            nc.sync.dma_start(out=ones_lo, in_=ones_d.ap()[0:128, :])
            nc.sync.dma_start(out=ones_hi, in_=ones_d.ap()[128:160, :])
            cidx = constp.tile([NCAP, 1], F32)
            nc.sync.dma_start(out=cidx, in_=cidx_d.ap())

            cb_sb = constp.tile([128, 2], F32)
            nc.sync.dma_start(out=cb_sb,
                              in_=conv_b.ap().rearrange("(t p) -> p t", p=128))
            pb_sb = constp.tile([128, 2], F32)
            nc.sync.dma_start(out=pb_sb,
                              in_=pc_b.ap().rearrange("(t p) -> p t", p=128))
            b1_sb = constp.tile([128, 4], F32)
            nc.sync.dma_start(out=b1_sb,
                              in_=b1.ap().rearrange("(t p) -> p t", p=128))
            b2_sb = constp.tile([128, 8], F32)
            nc.sync.dma_start(out=b2_sb,
                              in_=b2.ap().rearrange("(t p) -> p t", p=128))
            b3_sb = constp.tile([128, 7], F32)
            nc.vector.memset(b3_sb, 0.0)
            for t7 in range(6):
                nc.sync.dma_start(
                    out=b3_sb[:, t7:t7 + 1],
                    in_=b3.ap()[t7 * 128:(t7 + 1) * 128].rearrange("(p one) -> p one", one=1))
            nc.sync.dma_start(out=b3_sb[0:16, 6:7],
                              in_=b3.ap()[768:784].rearrange("(p one) -> p one", one=1))

            sT_lo = constp.tile([128, BL], F32)
            sT_hi = constp.tile([32, BL], F32)
            g_bcast = constp.tile([128, 1], F32)
            g8_bcast = constp.tile([128, 1], F32)
            gate = constp.tile([NCAP, BL], F32)

            Wperm_d = dramp.tile([KT, 128, 160], F32)
            WA_d = dramp.tile([160, FLAT], F32)
            F_d = dramp.tile([128, 4, R], F32)
            cc_in = [dramp.tile([1, 1], F32, name=f"cc_in{i}") for i in range(3)]
            cc_out = [dramp.tile([1, 1], F32, name=f"cc_out{i}") for i in range(3)]
            ccS_in = dramp.tile([1, NCAP], F32)
            ccS_out = dramp.tile([1, NCAP], F32)

            # =============== W_dc prep ===============
            with tc.tile_pool(name="wprep", bufs=1) as wp:
                wdc_nat = wp.tile([128, 9, 1280], F32)
                for rt in range(9):
                    nc.sync.dma_start(out=wdc_nat[:, rt, :],
                                      in_=wdc_f[rt * 128:(rt + 1) * 128, :])
                wdc_v = wdc_nat.rearrange("p rt (c o i) -> p rt c o i",
                                               c=NCAP, o=O16)
                wperm_sb = wp.tile([128, KT, 160], F32)
                for i in range(I8):
                    for rt in range(9):
                        nc.scalar.copy(out=wperm_sb[:, i * 9 + rt, :],
                                       in_=wdc_v[:, rt, :, :, i])
                for t in range(KT):
                    nc.sync.dma_start(out=Wperm_d[t], in_=wperm_sb[:, t, :])
                wa_lo = wp.tile([128, FLAT], F32)
                wa_hi = wp.tile([32, FLAT], F32)
                wa_lo_v = wa_lo.rearrange("p (rr q i) -> p rr q i", rr=9, i=8)
                wa_hi_v = wa_hi.rearrange("p (rr q i) -> p rr q i", rr=9, i=8)
                for t in range(KT):
                    i, rt = t // 9, t % 9
                    ps1 = psA.tile([128, 128], F32, tag="tp")
                    nc.tensor.transpose(ps1, wperm_sb[:, t, 0:128], ident)
                    nc.vector.tensor_copy(out=wa_lo_v[:, rt, :, i], in_=ps1)
                    ps2 = psA.tile([128, 128], F32, tag="tp")
                    nc.tensor.transpose(ps2[0:32, :], wperm_sb[:, t, 128:160], ident)
                    nc.vector.tensor_copy(out=wa_hi_v[:, rt, :, i], in_=ps2[0:32, :])
                nc.sync.dma_start(out=WA_d[0:128, :], in_=wa_lo)
                nc.sync.dma_start(out=WA_d[128:160, :], in_=wa_hi)

            # =============== convs ===============
            with tc.tile_pool(name="conv", bufs=1) as cp, \
                 tc.tile_pool(name="cps1", bufs=1) as cps1, \
                 tc.tile_pool(name="cps2", bufs=2) as cps2:
                cw_nat = cp.tile([128, 2, 81], F32)
                for t in range(2):
                    nc.sync.dma_start(out=cw_nat[:, t, :],
                                      in_=cw_f[t * 128:(t + 1) * 128, :])
                w1c = cp.tile([128, 2, 128], F32)
                for t in range(2):
                    psw = psA.tile([128, 128], F32, tag="tp")
                    nc.tensor.transpose(psw[0:81, :], cw_nat[:, t, :], ident)
                    nc.vector.tensor_copy(out=w1c[0:81, t, :], in_=psw[0:81, :])

                F0 = cp.tile([128, 4, R], F32, tag="F0")
                F1 = cp.tile([128, 4, R], F32, tag="F1")
                pcwT = cp.tile([128, 81, 2, 128], F32)

                for ph in range(2):
                    Fph = F0 if ph == 0 else F1
                    Fv = Fph.rearrange("(cl b) t (c q) -> cl b t c q",
                                            cl=2, c=32)
                    for oc_t in range(2):
                        for sl in range(3):
                            slab = cps1.tile([128, 128, 27], F32, tag="pcwslab")
                            src = bass.AP(
                                tensor=pcw_f.tensor,
                                offset=(pcw_f.offset + (oc_t * 128) * 20736
                                        + ph * 128 * 81 + sl * 27),
                                ap=[[20736, 128], [81, 128], [1, 27]])
                            nc.sync.dma_start(out=slab, in_=src)
                            for dl in range(27):
                                pst = psA.tile([128, 128], F32, tag="tp")
                                nc.tensor.transpose(pst, slab[:, :, dl], ident)
                                nc.vector.tensor_copy(
                                    out=pcwT[:, sl * 27 + dl, oc_t, :], in_=pst)
                    for ch in range(NCHUNK):
                        patches = cps1.tile([128, S, 20, 20], F32, tag="patches")
                        for dy in range(9):
                            for dx in range(9):
                                src = bass.AP(
                                    tensor=data_f.tensor,
                                    offset=(data_f.offset + (ch * S) * 784
                                            + dy * 28 + dx),
                                    ap=[[784, S], [28, 20], [1, 20]])
                                nc.sync.dma_start(
                                    out=patches[dy * 9 + dx:dy * 9 + dx + 1, :, :, :],
                                    in_=src)
                        xch = cps2.tile([128, S, 20, 20], F32, tag="xch")
                        pflat = patches.rearrange("p s h w -> p (s h w)")
                        xflat = xch.rearrange("p s h w -> p (s h w)")
                        xpar = xch.rearrange("p s (y2 ty) (x2 tx) -> p s y2 ty x2 tx",
                                             ty=2, tx=2)
                        NTOT = S * 400
                        for nch in range((NTOT + 511) // 512):
                            n0 = nch * 512
                            nn = min(512, NTOT - n0)
                            pcv = psB.tile([128, 512], F32, tag="bigmm")
                            nc.tensor.matmul(pcv[:, 0:nn], lhsT=w1c[0:81, ph, :],
                                             rhs=pflat[0:81, n0:n0 + nn],
                                             start=True, stop=True)
                            nc.scalar.activation(out=xflat[:, n0:n0 + nn],
                                                 in_=pcv[:, 0:nn], func=AF.Relu,
                                                 bias=cb_sb[:, ph:ph + 1], scale=1.0)
                        for oc_t in range(2):
                            pp = psD.tile([128, 512], F32, tag="acc")
                            for dy in range(9):
                                for dx in range(9):
                                    dd = dy * 9 + dx
                                    rhs = xpar[:, :, dy // 2:dy // 2 + 6,
                                               dy % 2, dx // 2:dx // 2 + 6, dx % 2]
                                    nc.tensor.matmul(
                                        pp[:, 0:S * 36], lhsT=pcwT[:, dd, oc_t, :],
                                        rhs=rhs, start=(dd == 0), stop=(dd == 80))
                            utmp = cps2.tile([128, S, 36], F32, tag="utmp")
                            uflat = utmp.rearrange("p s q -> p (s q)")
                            nc.scalar.copy(out=uflat, in_=pp[:, 0:S * 36])
                            if ph == 1:
                                nc.vector.tensor_scalar_add(
                                    out=uflat, in0=uflat,
                                    scalar1=pb_sb[:, oc_t:oc_t + 1])
                            for s in range(S):
                                bg = ch * S + s
                                for half in range(2):
                                    tt = oc_t * 2 + half
                                    nc.sync.dma_start(
                                        out=Fv[:, bg, tt, :, :],
                                        in_=utmp[half * 64:(half + 1) * 64, s, :])

                # --- squash ---
                F0f = F0.rearrange("p t f -> p (t f)")
                F1f = F1.rearrange("p t f -> p (t f)")
                nc.vector.tensor_add(out=F0f, in0=F0f, in1=F1f)
                Fsq = cp.tile([128, 4, R], F32, tag="F1")   # reuse F1 slot
                Fsqf = Fsq.rearrange("p t f -> p (t f)")
                nc.scalar.square(out=Fsqf, in_=F0f)
                sqg = cp.tile([128, 576], F32)
                nc.vector.reduce_sum(
                    out=sqg,
                    in_=Fsq.rearrange("p t (g i) -> p (t g) i", i=8),
                    axis=mybir.AxisListType.X)
                sig = cp.tile([128, 576], F32)
                nc.scalar.sqrt(out=sig, in_=sqg)
                sq1 = cp.tile([128, 576], F32)
                nc.scalar.add(out=sq1, in_=sqg, add=1.0)
                nc.vector.tensor_mul(out=sig, in0=sig, in1=sq1)
                nc.vector.reciprocal(out=sig, in_=sig)
                nc.vector.tensor_mul(out=sig, in0=sig, in1=sqg)
                F0i = F0.rearrange("p t (g i) -> p t g i", i=8)
                sigv = sig.rearrange("p (t g) -> p t g", t=4)
                for i in range(8):
                    nc.vector.tensor_tensor(out=F0i[:, :, :, i], in0=F0i[:, :, :, i],
                                            in1=sigv, op=ALU.mult)
                nc.sync.dma_start(out=F_d, in_=F0)
                if DEBUG:
                    nc.sync.dma_start(out=dbg["F"].ap(), in_=F0)

            # =============== routing ===============
            with tc.tile_pool(name="route", bufs=1) as rp, \
                 tc.tile_pool(name="routes", bufs=2) as rps, \
                 tc.tile_pool(name="rp4", bufs=4) as rp4:
                Fst = rp.tile([128, 4, R], F32)
                nc.sync.dma_start(out=Fst, in_=F_d)
                u_dup = rp.tile([128, FLAT], F32)
                ud_v = u_dup.rearrange("(c2 b) (tp clp f) -> c2 b tp clp f",
                                            c2=2, tp=4, clp=2)
                Fd_v = F_d.rearrange("(cl b) t f -> cl b t f", cl=2)
                for c2 in range(2):
                    for cl in range(2):
                        nc.sync.dma_start(out=ud_v[c2, :, :, cl, :],
                                          in_=Fd_v[cl, :, :, :])
                uT = rp.tile([128, KT, BL], F32)
                uT_v = uT.rearrange("p (i rr) b -> p i rr b", i=8)
                Fst_v = Fst.rearrange("(cl b) t (fc fr) -> cl b t fc fr",
                                           cl=2, fc=9)
                for cap in range(8):
                    tt, cl = cap // 2, cap % 2
                    for fch in range(9):
                        pst = psA.tile([128, 128], F32, tag="tp")
                        nc.tensor.transpose(pst[:, 0:64],
                                            Fst_v[cl, :, tt, fch, :],
                                            ident[cl * 64:(cl + 1) * 64,
                                                  cl * 64:(cl + 1) * 64])
                        ustage = rps.tile([128, BL], F32, tag="uTstage")
                        nc.scalar.copy(out=ustage, in_=pst[:, 0:64])
                        q = cap * 144 + fch * 16
                        qd, qp = q // 128, q % 128
                        ust_v = ustage.rearrange("(f8 i) b -> i f8 b", i=8)
                        for i in range(8):
                            nc.sync.dma_start(out=uT_v[qp:qp + 16, i, qd, :],
                                              in_=ust_v[i])
                if DEBUG:
                    nc.sync.dma_start(out=dbg["udup"].ap(), in_=u_dup)
                    nc.sync.dma_start(out=dbg["uT"].ap(), in_=uT)

                b_ij = rp.tile([128, 5, R], F32)
                nc.vector.memset(b_ij.rearrange("p q r -> p (q r)"), 0.0)
                cT = rp.tile([128, 9, 640], F32)

                for it in range(3):
                    # ---- sT ----
                    if it == 0:
                        psS0 = psB.tile([128, 512], F32, tag="bigmm")
                        for t in range(KT):
                            wpt = rp4.tile([128, 160], F32, tag="wperm")
                            nc.sync.dma_start(out=wpt, in_=Wperm_d[t])
                            nc.tensor.matmul(psS0[0:64, 0:160], lhsT=uT[:, t, :],
                                             rhs=wpt, start=(t == 0),
                                             stop=(t == KT - 1))
                        s0sb = rps.tile([64, 160], F32, tag="s0sb")
                        nc.scalar.activation(out=s0sb, in_=psS0[0:64, 0:160],
                                             func=AF.Copy, bias=0.0, scale=1.0 / R)
                        pst = psA.tile([128, 128], F32, tag="tp")
                        nc.tensor.transpose(pst[:, 0:64], s0sb[:, 0:128],
                                            ident[0:64, 0:64])
                        nc.vector.tensor_copy(out=sT_lo, in_=pst[:, 0:64])
                        pst2 = psA.tile([128, 128], F32, tag="tp")
                        nc.tensor.transpose(pst2[0:32, 0:64], s0sb[:, 128:160],
                                            ident[0:64, 0:64])
                        nc.vector.tensor_copy(out=sT_hi, in_=pst2[0:32, 0:64])
                    else:
                        spair = rps.tile([128, 160], F32, tag="spair")
                        for pi in range(5):
                            psS = psB.tile([128, 512], F32, tag="bigmm")
                            for t in range(KT):
                                rc = t % 9
                                wpt = rp4.tile([128, 32], F32, tag="wperm")
                                nc.sync.dma_start(
                                    out=wpt,
                                    in_=Wperm_d[t][:, pi * 32:(pi + 1) * 32])
                                Xt = rp4.tile([128, 128], F32, tag="X")
                                for cl in range(2):
                                    nc.vector.tensor_tensor(
                                        out=Xt[:, cl * 64:(cl + 1) * 64],
                                        in0=uT[:, t, :],
                                        in1=cT[:, rc, pi * 128 + cl * 64:
                                               pi * 128 + (cl + 1) * 64],
                                        op=ALU.mult)
                                nc.tensor.matmul(
                                    psS[:, 0:32], lhsT=Xt,
                                    rhs=wpt,
                                    start=(t == 0), stop=(t == KT - 1))
                            nc.scalar.copy(out=spair[:, pi * 32:(pi + 1) * 32],
                                           in_=psS[:, 0:32])
                        for pi in range(5):
                            for cl in range(2):
                                c = pi * 2 + cl
                                pst = psA.tile([128, 128], F32, tag="tp")
                                nc.tensor.transpose(
                                    pst[0:16, 0:64],
                                    spair[cl * 64:(cl + 1) * 64,
                                          pi * 32 + cl * 16:pi * 32 + cl * 16 + 16],
                                    ident[cl * 64:(cl + 1) * 64,
                                          cl * 64:(cl + 1) * 64])
                                s16 = rps.tile([16, BL], F32, tag="s16")
                                nc.scalar.copy(out=s16, in_=pst[0:16, 0:64])
                                if c < 8:
                                    nc.sync.dma_start(
                                        out=sT_lo[c * 16:(c + 1) * 16, :],
                                        in_=s16)
                                else:
                                    nc.sync.dma_start(
                                        out=sT_hi[(c - 8) * 16:(c - 7) * 16, :],
                                        in_=s16)
                    if DEBUG:
                        nc.sync.dma_start(out=dbg["sT"].ap()[it, 0:128, :],
                                          in_=sT_lo)
                        nc.sync.dma_start(out=dbg["sT"].ap()[it, 128:160, :],
                                          in_=sT_hi)

                    # ---- global sq -> g ----
                    acc1 = rps.tile([128, 1], F32, tag="acc")
                    acc2 = rps.tile([32, 1], F32, tag="acc2")
                    sqtmp = rps.tile([128, BL], F32, tag="sqtmp")
                    nc.scalar.activation(out=sqtmp, in_=sT_lo, func=AF.Square,
                                         bias=0.0, scale=1.0, accum_out=acc1)
                    nc.scalar.activation(out=sqtmp[0:32, :], in_=sT_hi,
                                         func=AF.Square, bias=0.0, scale=1.0,
                                         accum_out=acc2)
                    nc.vector.tensor_add(out=acc1[0:32, :], in0=acc1[0:32, :],
                                         in1=acc2)
                    nc.gpsimd.partition_all_reduce(acc1, acc1, channels=128,
                                                   reduce_op=bass_isa.ReduceOp.add)
                    nc.sync.dma_start(out=cc_in[it], in_=acc1[0:1, :])
                    nc.gpsimd.collective_compute(
                        "AllReduce", ALU.add,
                        replica_groups=[list(range(NCORES))],
                        ins=[cc_in[it].opt()], outs=[cc_out[it].opt()])
                    sqs = rps.tile([1, 4], F32, tag="sqs")
                    nc.sync.dma_start(out=sqs[:, 0:1], in_=cc_out[it])
                    nc.scalar.sqrt(out=sqs[:, 1:2], in_=sqs[:, 0:1])
                    nc.scalar.add(out=sqs[:, 2:3], in_=sqs[:, 0:1], add=1.0)
                    nc.vector.tensor_mul(out=sqs[:, 1:2], in0=sqs[:, 1:2],
                                         in1=sqs[:, 2:3])
                    nc.vector.reciprocal(out=sqs[:, 1:2], in_=sqs[:, 1:2])
                    nc.vector.tensor_mul(out=sqs[:, 3:4], in0=sqs[:, 0:1],
                                         in1=sqs[:, 1:2])
                    nc.gpsimd.partition_broadcast(g_bcast, sqs[0:1, 3:4])
                    nc.vector.tensor_scalar(out=g8_bcast, in0=g_bcast, scalar1=8.0,
                                            scalar2=None, op0=ALU.mult)
                    if DEBUG:
                        nc.sync.dma_start(out=dbg["g"].ap()[it:it + 1, :],
                                          in_=sqs[0:1, 3:4])

                    if it == 2:
                        break

                    # ---- A-side ----
                    for pi in range(5):
                        vbd = rps.tile([32, 128], F32, tag="vbd")
                        nc.vector.memset(vbd, 0.0)
                        for cl in range(2):
                            c = pi * 2 + cl
                            if c < 8:
                                src = sT_lo[c * 16:(c + 1) * 16, :]
                            else:
                                src = sT_hi[(c - 8) * 16:(c - 7) * 16, :]
                            nc.sync.dma_start(
                                out=vbd[cl * 16:(cl + 1) * 16,
                                        cl * 64:(cl + 1) * 64],
                                in_=src)
                        Apair = rps.tile([128, R], F32, tag="Apair")
                        for nch2 in range(FLAT // 512):
                            n0 = nch2 * 512
                            wa_c = rp4.tile([32, 512], F32, tag="wa")
                            nc.sync.dma_start(
                                out=wa_c, in_=WA_d[pi * 32:(pi + 1) * 32,
                                                   n0:n0 + 512])
                            psQ = psB.tile([128, 512], F32, tag="bigmm")
                            nc.tensor.matmul(psQ, lhsT=vbd.bitcast(F32R),
                                             rhs=wa_c.bitcast(F32R),
                                             start=True, stop=True)
                            Pm = rp4.tile([128, 512], F32, tag="Pm")
                            nc.vector.tensor_mul(out=Pm, in0=psQ,
                                                 in1=u_dup[:, n0:n0 + 512])
                            Pm_v = Pm.rearrange("p (r i) -> p r i", i=8)
                            nc.vector.reduce_sum(
                                out=Apair[:, nch2 * 64:(nch2 + 1) * 64],
                                in_=Pm_v, axis=mybir.AxisListType.X)
                        nc.vector.scalar_tensor_tensor(
                            out=b_ij[:, pi, :], in0=Apair, scalar=g_bcast,
                            in1=b_ij[:, pi, :], op0=ALU.mult, op1=ALU.add)
                    if DEBUG:
                        nc.sync.dma_start(out=dbg["bij"].ap()[it], in_=b_ij)

                    # ---- softmax over routes + cT ----
                    for pi in range(5):
                        nmax = rps.tile([128, 1], F32, tag="nmax")
                        nc.vector.tensor_reduce(out=nmax, in_=b_ij[:, pi, :],
                                                axis=mybir.AxisListType.X,
                                                op=ALU.max, negate=True)
                        et = rps.tile([128, R], F32, tag="et")
                        sume = rps.tile([128, 1], F32, tag="sume")
                        nc.scalar.activation(out=et, in_=b_ij[:, pi, :],
                                             func=AF.Exp, bias=nmax, scale=1.0,
                                             accum_out=sume)
                        nc.vector.reciprocal(out=sume, in_=sume)
                        nc.vector.tensor_scalar_mul(out=et, in0=et, scalar1=sume)
                        for rc in range(9):
                            psc = psA.tile([128, 128], F32, tag="tp")
                            nc.tensor.transpose(psc, et[:, rc * 128:(rc + 1) * 128],
                                                ident)
                            nc.scalar.copy(out=cT[:, rc, pi * 128:(pi + 1) * 128],
                                           in_=psc)

            # =============== classes / argmax / decoder / outputs ===============
            with tc.tile_pool(name="dec", bufs=1) as dp, \
                 tc.tile_pool(name="decs", bufs=2) as dps:
                vT_lo = dps.tile([128, BL], F32, tag="vtl")
                vT_hi = dps.tile([32, BL], F32, tag="vth")
                nc.vector.tensor_scalar_mul(out=vT_lo, in0=sT_lo, scalar1=g_bcast)
                nc.vector.tensor_scalar_mul(out=vT_hi, in0=sT_hi,
                                            scalar1=g_bcast[0:32, :])
                sqlo = dps.tile([128, BL], F32, tag="sql")
                sqhi = dps.tile([32, BL], F32, tag="sqh")
                nc.scalar.square(out=sqlo, in_=vT_lo)
                nc.scalar.square(out=sqhi, in_=vT_hi)
                psN = psD.tile([128, 512], F32, tag="acc")
                nc.tensor.matmul(psN[0:NCAP, 0:BL], lhsT=ones_lo, rhs=sqlo,
                                 start=True, stop=False)
                nc.tensor.matmul(psN[0:NCAP, 0:BL], lhsT=ones_hi, rhs=sqhi,
                                 start=False, stop=True)
                cls = dps.tile([NCAP, BL], F32, tag="cls")
                nc.scalar.sqrt(out=cls, in_=psN[0:NCAP, 0:BL])
                if DEBUG:
                    nc.sync.dma_start(out=dbg["cls"].ap(), in_=cls)
                e10 = dps.tile([NCAP, BL], F32, tag="e10")
                Spart = dps.tile([NCAP, 1], F32, tag="Sp")
                nc.scalar.activation(out=e10, in_=cls, func=AF.Exp, bias=0.0,
                                     scale=1.0, accum_out=Spart)
                nc.sync.dma_start(out=ccS_in, in_=Spart)
                nc.gpsimd.collective_compute(
                    "AllReduce", ALU.add, replica_groups=[list(range(NCORES))],
                    ins=[ccS_in.opt()], outs=[ccS_out.opt()])
                Sg = dps.tile([NCAP, 1], F32, tag="Sg")
                nc.sync.dma_start(out=Sg, in_=ccS_out)
                nc.vector.reciprocal(out=Sg, in_=Sg)
                nc.vector.tensor_scalar_mul(out=e10, in0=e10, scalar1=Sg)
                if DEBUG:
                    nc.sync.dma_start(out=dbg["q"].ap(), in_=e10)
                qm = dps.tile([NCAP, BL], F32, tag="qm")
                nc.gpsimd.partition_all_reduce(qm, e10, channels=NCAP,
                                               reduce_op=bass_isa.ReduceOp.max)
                # near-max band (rel 6e-7), lowest index wins (matches jax
                # argmax first-max tie semantics at fp32 noise level)
                thr = dps.tile([NCAP, BL], F32, tag="thr")
                nc.vector.tensor_scalar_mul(out=thr, in0=qm,
                                            scalar1=1.0 - 2.2e-7)
                near = dps.tile([NCAP, BL], F32, tag="near")
                nc.vector.tensor_tensor(out=near, in0=e10, in1=thr, op=ALU.is_ge)
                cand = dps.tile([NCAP, BL], F32, tag="cand")
                # cand = near ? -c : -1e9   (then max over partitions, equality)
                nc.vector.tensor_scalar(out=cand, in0=near, scalar1=cidx,
                                        scalar2=None, op0=ALU.mult)
                pen = dps.tile([NCAP, BL], F32, tag="pen")
                nc.vector.tensor_scalar(out=pen, in0=near, scalar1=1e9,
                                        scalar2=1e9, op0=ALU.mult, op1=ALU.subtract)
                # pen = near*1e9 - 1e9  (0 if near, -1e9 if not)
                nc.vector.tensor_sub(out=cand, in0=pen, in1=cand)
                # cand = pen - near*c = -c if near else -1e9
                cmax = dps.tile([NCAP, BL], F32, tag="cmax")
                nc.gpsimd.partition_all_reduce(cmax, cand, channels=NCAP,
                                               reduce_op=bass_isa.ReduceOp.max)
                nc.vector.tensor_tensor(out=gate, in0=cand, in1=cmax,
                                        op=ALU.is_equal)
                psg = psA.tile([128, 128], F32, tag="tp")
                nc.tensor.transpose(psg[0:BL, 0:NCAP], gate,
                                    ident[0:NCAP, 0:NCAP])
                gT = dps.tile([BL, NCAP], F32, tag="gT")
                nc.vector.tensor_copy(out=gT, in_=psg[0:BL, 0:NCAP])
                nc.sync.dma_start(out=out_masked.ap(), in_=gT)
                # gate_dup over o via DRAM-bounce broadcast DMA
                gate_d = dramp.tile([NCAP, BL], F32)
                nc.sync.dma_start(out=gate_d, in_=gate)
                gdup_lo = dps.tile([128, BL], F32, tag="gdl")
                gdup_hi = dps.tile([32, BL], F32, tag="gdh")
                gsrc_lo = bass.AP(tensor=gate_d.tensor,
                                  offset=gate_d.offset,
                                  ap=[[BL, 8], [0, 16], [1, BL]])
                gsrc_hi = bass.AP(tensor=gate_d.tensor,
                                  offset=gate_d.offset + 8 * BL,
                                  ap=[[BL, 2], [0, 16], [1, BL]])
                nc.sync.dma_start(out=gdup_lo, in_=gsrc_lo)
                nc.sync.dma_start(out=gdup_hi, in_=gsrc_hi)
                tT_lo = dps.tile([128, BL], F32, tag="ttl")
                tT_hi = dps.tile([32, BL], F32, tag="tth")
                nc.vector.tensor_mul(out=tT_lo, in0=sT_lo, in1=gdup_lo)
                nc.vector.tensor_mul(out=tT_hi, in0=sT_hi, in1=gdup_hi)

                # ---- v output ----
                vout = dp.tile([BL, 160], F32)
                psv1 = psA.tile([128, 128], F32, tag="tp")
                nc.tensor.transpose(psv1[0:64, :], vT_lo, ident)
                nc.vector.tensor_copy(out=vout[:, 0:128], in_=psv1[0:64, :])
                psv2 = psA.tile([128, 128], F32, tag="tp")
                nc.tensor.transpose(psv2[0:64, 0:32], vT_hi, ident[0:32, 0:32])
                nc.vector.tensor_copy(out=vout[:, 128:160], in_=psv2[0:64, 0:32])
                nc.sync.dma_start(out=outv_f, in_=vout)

                # ---- decoder ----
                w1T = dp.tile([128, 2, 4, 128], F32)
                w1nat = dps.tile([128, 4, 160], F32, tag="w1nat")
                for t4 in range(4):
                    nc.sync.dma_start(out=w1nat[:, t4, :],
                                      in_=w1.ap()[t4 * 128:(t4 + 1) * 128, :])
                for t4 in range(4):
                    ps1 = psA.tile([128, 128], F32, tag="tp")
                    nc.tensor.transpose(ps1, w1nat[:, t4, 0:128], ident)
                    nc.vector.tensor_copy(out=w1T[:, 0, t4, :], in_=ps1)
                    ps2 = psA.tile([128, 128], F32, tag="tp")
                    nc.tensor.transpose(ps2[0:32, :], w1nat[:, t4, 128:160], ident)
                    nc.vector.tensor_copy(out=w1T[0:32, 1, t4, :], in_=ps2[0:32, :])
                h1 = dp.tile([128, 4, BL], F32)
                for m4 in range(4):
                    psh = psD.tile([128, 512], F32, tag="acc")
                    nc.tensor.matmul(psh[:, 0:BL], lhsT=w1T[:, 0, m4, :],
                                     rhs=tT_lo, start=True, stop=False)
                    nc.tensor.matmul(psh[:, 0:BL], lhsT=w1T[0:32, 1, m4, :],
                                     rhs=tT_hi, start=False, stop=True)
                    nc.scalar.activation(out=h1[:, m4, :], in_=psh[:, 0:BL],
                                         func=AF.Relu, bias=b1_sb[:, m4:m4 + 1],
                                         scale=g_bcast)
                w2T = dp.tile([128, 4, 8, 128], F32)
                for t8 in range(8):
                    w2nat = dps.tile([128, 512], F32, tag="w2nat")
                    nc.sync.dma_start(out=w2nat,
                                      in_=w2.ap()[t8 * 128:(t8 + 1) * 128, :])
                    for k4 in range(4):
                        psw = psA.tile([128, 128], F32, tag="tp")
                        nc.tensor.transpose(psw, w2nat[:, k4 * 128:(k4 + 1) * 128],
                                            ident)
                        nc.vector.tensor_copy(out=w2T[:, k4, t8, :], in_=psw)
                h2 = dp.tile([128, 8, BL], F32)
                for m8 in range(8):
                    psh = psD.tile([128, 512], F32, tag="acc")
                    for k4 in range(4):
                        nc.tensor.matmul(psh[:, 0:BL], lhsT=w2T[:, k4, m8, :],
                                         rhs=h1[:, k4, :],
                                         start=(k4 == 0), stop=(k4 == 3))
                    nc.scalar.activation(out=h2[:, m8, :], in_=psh[:, 0:BL],
                                         func=AF.Relu, bias=b2_sb[:, m8:m8 + 1],
                                         scale=1.0)
                w3T = dp.tile([128, 8, 784], F32)
                for t7 in range(7):
                    pt = 128 if t7 < 6 else 16
                    w3nat = dps.tile([128, 1024], F32, tag="w3nat")
                    nc.sync.dma_start(out=w3nat[0:pt, :],
                                      in_=w3.ap()[t7 * 128:t7 * 128 + pt, :])
                    for k8 in range(8):
                        psw = psA.tile([128, 128], F32, tag="tp")
                        nc.tensor.transpose(psw[:, 0:pt],
                                            w3nat[0:pt, k8 * 128:(k8 + 1) * 128],
                                            ident[0:pt, 0:pt])
                        nc.vector.tensor_copy(out=w3T[:, k8, t7 * 128:t7 * 128 + pt],
                                              in_=psw[:, 0:pt])
                recT = dp.tile([BL, 784], F32)
                for t7 in range(7):
                    pt = 128 if t7 < 6 else 16
                    psh = psD.tile([128, 512], F32, tag="acc")
                    for k8 in range(8):
                        nc.tensor.matmul(psh[0:pt, 0:BL],
                                         lhsT=w3T[:, k8, t7 * 128:t7 * 128 + pt],
                                         rhs=h2[:, k8, :],
                                         start=(k8 == 0), stop=(k8 == 7))
                    rsb = dps.tile([128, BL], F32, tag="rsb")
                    nc.scalar.activation(out=rsb[0:pt, :], in_=psh[0:pt, 0:BL],
                                         func=AF.Sigmoid,
                                         bias=b3_sb[0:pt, t7:t7 + 1], scale=1.0)
                    psr = psA.tile([128, 128], F32, tag="tp")
                    nc.tensor.transpose(psr[0:64, 0:pt], rsb[0:pt, :],
                                        ident[0:pt, 0:pt])
                    nc.vector.tensor_copy(out=recT[:, t7 * 128:t7 * 128 + pt],
                                          in_=psr[0:64, 0:pt])
                nc.sync.dma_start(out=outr_f, in_=recT)

    nc.compile()
    return nc


def kernel(**inputs):
    if "nc" not in _CACHE:
        _CACHE["nc"] = build_nc()
    nc = _CACHE["nc"]
    data = np.ascontiguousarray(inputs["data"], dtype=np.float32)
    wnames = ["conv_w", "conv_b", "pc_w", "pc_b", "W_dc",
              "w1", "b1", "w2", "b2", "w3", "b3"]
    in_maps = []
    for k in range(NCORES):
        m = {"data": np.ascontiguousarray(data[k * BL:(k + 1) * BL])}
        for w in wnames:
            m[w] = np.ascontiguousarray(inputs[w], dtype=np.float32)
        in_maps.append(m)
    trace = bool(int(os.environ.get("CAPS_TRACE", "0")))
    res = bass_utils.run_bass_kernel_spmd(nc, in_maps, core_ids=list(range(NCORES)),
                                          trace=trace)
    _CACHE["last"] = res
    v = np.concatenate([r["out_v"] for r in res.results], 0)
    recon = np.concatenate([r["out_recon"] for r in res.results], 0)
    masked = np.concatenate([r["out_masked"] for r in res.results], 0)
    return v, recon, masked
